# revision 1
# baseline (speedup 1.0000x reference)
"""Attention-LSTM decoder kernel for Trainium2, data-parallel over batch on 8 NeuronCores.

Strategy (per sharding hint): batch_H/text sharded on dim 0 across the 8 cores,
all parameters replicated. The recurrence is fully independent per batch element,
so no collectives are needed; results are concatenated on the host.

The embedding gather (emb[text]) is an int-indexed host-side layout step: it
shrinks the device problem (no 6.8 MB emb table on device, no int64 gather) and
feeds the per-step char embeddings directly.
"""
import numpy as np
from functools import partial

import jax
import jax.numpy as jnp

# Problem shapes (hardcoded per spec nn_Attention_69758858822101)
B, T, D, H, E, C, S = 256, 64, 512, 512, 256, 6624, 26
NCORES = 8
BS = B // NCORES  # 32 per-core batch


def _core_fn(batch_H, ce, W_i2h, W_h2h, b_h2h, w_score, W_ih, W_hh, b_ih, b_hh,
             W_gen, b_gen):
    """Per-core computation: batch shard [BS,T,D] + gathered char embs [BS,S,E]."""
    Hproj = jnp.einsum('btd,hd->bth', batch_H, W_i2h)      # [BS,T,H]
    xs = jnp.transpose(ce, (1, 0, 2))                       # [S,BS,E]

    def step(carry, x):
        h, c = carry
        hp = h @ W_h2h.T + b_h2h                            # [BS,H]
        e = jnp.tanh(Hproj + hp[:, None, :]) @ w_score      # [BS,T]
        alpha = jax.nn.softmax(e, axis=1)
        context = jnp.einsum('bt,btd->bd', alpha, batch_H)  # [BS,D]
        xx = jnp.concatenate([context, x], axis=1)          # [BS,D+E]
        gates = xx @ W_ih.T + b_ih + h @ W_hh.T + b_hh      # [BS,4H]
        i, f, g, o = jnp.split(gates, 4, axis=1)
        c_new = jax.nn.sigmoid(f) * c + jax.nn.sigmoid(i) * jnp.tanh(g)
        h_new = jax.nn.sigmoid(o) * jnp.tanh(c_new)
        return (h_new, c_new), h_new

    h0 = jnp.zeros((batch_H.shape[0], H), jnp.float32)
    c0 = jnp.zeros_like(h0)
    _, hs = jax.lax.scan(step, (h0, c0), xs)                # [S,BS,H]
    oh = jnp.transpose(hs, (1, 0, 2))                       # [BS,S,H]
    return oh @ W_gen.T + b_gen                             # [BS,S,C]


_pmapped = None


def _get_pmapped():
    global _pmapped
    if _pmapped is None:
        _pmapped = jax.pmap(
            _core_fn,
            in_axes=(0, 0) + (None,) * 10,
            devices=jax.devices()[:NCORES],
        )
    return _pmapped


def kernel(batch_H, text, W_i2h, W_h2h, b_h2h, w_score, W_ih, W_hh, b_ih, b_hh,
           emb, W_gen, b_gen, max_label_length):
    batch_H = np.asarray(batch_H, np.float32)
    text = np.asarray(text)
    emb = np.asarray(emb, np.float32)
    num_steps = int(max_label_length) + 1

    # Host-side gather of char embeddings (index layout step), then shard.
    ce = emb[text[:, :num_steps].astype(np.int64)]          # [B,S,E]
    bh_sh = batch_H.reshape(NCORES, BS, T, D)
    ce_sh = ce.reshape(NCORES, BS, num_steps, E)

    params = [np.asarray(p, np.float32) for p in
              (W_i2h, W_h2h, b_h2h, w_score, W_ih, W_hh, b_ih, b_hh, W_gen, b_gen)]

    out = _get_pmapped()(bh_sh, ce_sh, *params)             # [NCORES,BS,S,C]
    out = np.asarray(out, np.float32).reshape(B, num_steps, C)
    return out
